# revision 10
# baseline (speedup 1.0000x reference)
"""Trainium2 Bass kernel for nn_BlockSparseMoE (top-2 of 8 experts, SwiGLU).

Strategy (chunk-scheduled expert parallelism):
  - Host: compute router (gate matmul + softmax + top-2 + renorm) in fp64.
  - The device graph processes a fixed list of token-chunk slots per core
    (identical across cores).  Each slot streams its OWN w1/w3/w2 from
    DRAM, so the host is free to bind ANY expert's weights to any slot.
    Capacity therefore isn't max-expert-load (1129 for the reference
    router) but the optimum of a small bin-covering problem: slot sizes
    (496, 320, 240) x 8 cores cover the 8 expert loads with ~3% padding
    (sum 1056/core), vs 10% for one-expert-per-core.
  - Device (SPMD x8): per slot, hT[f, t] = silu(x@w1)^T * (x@w3)^T is
    computed directly transposed (phase A), then y[t, d] = hT^T @ w2
    scaled by the renormalized top-2 weight (phase B). bf16 in, fp32 PSUM.
  - Host: scatter-add the per-slot outputs back by token.

Per-core layout notes:
  - w1/w3 arrive host-pre-tiled [fg, p, dc, 512] so each f-group DMA is a
    contiguous 8KB line per partition; streamed per slot with a 3-deep
    issue-ahead queue (the 240-token slot consumes 2MB per 6.4us f-group
    window -- deeper prefetch rides through that burst).
  - w2 is ONE resident SBUF tile, re-filled per slot during phase A of
    that slot (phase B of slot t-1 releases it exactly when phase A of
    slot t starts on the PE, and phase A lasts >= 51us vs 22us of DMA).
"""

import numpy as np
import ml_dtypes

HIDDEN = 1024
FFN = 4096
NUM_EXPERTS = 8
TOP_K = 2
N_CORES = 8
N_WARM = 72

_BF16 = ml_dtypes.bfloat16
_nc_cache = {}


# ---------------------------------------------------------------- router ----
def _route(x, gate_w, gate_b):
    """Top-2 routing. Returns per-expert (token_idx, renorm_weight)."""
    logits = x.astype(np.float64) @ gate_w.astype(np.float64) + gate_b.astype(
        np.float64
    )
    logits -= logits.max(axis=-1, keepdims=True)
    p = np.exp(logits)
    p /= p.sum(axis=-1, keepdims=True)
    # top-2 by prob, ties broken by lower index (matches jax.lax.top_k)
    top2 = np.argsort(-p, axis=-1, kind="stable")[:, :TOP_K]
    pt = np.take_along_axis(p, top2, axis=-1)
    wt = pt / pt.sum(axis=-1, keepdims=True)
    idxs, wts = [], []
    for e in range(NUM_EXPERTS):
        mask = top2 == e  # [T, 2]
        tok = np.nonzero(mask.any(axis=-1))[0]
        w = wt[tok, np.argmax(mask[tok], axis=-1)]
        idxs.append(tok)
        wts.append(w.astype(np.float32))
    return idxs, wts


# ------------------------------------------------------------ slot planner --
def _try_cover(sizes, loads, node_budget=200000):
    """Can 8 instances of each slot size cover the loads (sum >= load each)?
    Returns per-load tuples (count per slot size) or None."""
    import itertools
    import functools

    ns = len(sizes)
    nodes = [0]

    @functools.lru_cache(maxsize=None)
    def combos_for(need, rem):
        out = []
        maxk = [min(rem[j], (need // sizes[j]) + 1) for j in range(ns)]
        for ks in itertools.product(*[range(k + 1) for k in maxk]):
            tot = sum(k * s for k, s in zip(ks, sizes))
            # minimal covers only (dropping any instance goes below need)
            if tot >= need and all(
                k == 0 or tot - s < need for k, s in zip(ks, sizes)
            ):
                out.append(ks)
        out.sort(key=lambda ks: sum(k * s for k, s in zip(ks, sizes)))
        return out

    result = []

    def dfs(ei, rem):
        nodes[0] += 1
        if nodes[0] > node_budget:
            return False
        if ei == len(loads):
            return True
        for ks in combos_for(loads[ei], rem):
            result.append(ks)
            if dfs(ei + 1, tuple(r - k for r, k in zip(rem, ks))):
                return True
            result.pop()
        return False

    if dfs(0, tuple([8] * ns)):
        return list(result)
    return None


def _capacity_chunks(max_load):
    """Fallback slot structure: per-core chunks covering one full expert
    (the original one-expert-per-core layout). Always feasible."""
    C = max_load
    n = max(1, -(-C // 512))
    chunks = []
    rem = C
    for i in range(n - 1):
        c = min(512, -(-rem // ((n - i) * 128)) * 128)
        chunks.append(c)
        rem -= c
    while n > 1 and rem < 240 and chunks:
        for i in range(len(chunks)):
            if rem >= 240:
                break
            if chunks[i] > 256:
                chunks[i] -= 128
                rem += 128
        else:
            break
    rem = -(-rem // 16) * 16  # pad ragged tail to /16 (padding cols, s=0)
    chunks.append(rem)
    return tuple(chunks)


def _plan(loads):
    """Choose slot sizes + per-expert slot counts for the 8-core fleet.

    Returns (sizes, assign) where assign[e] = tuple of instance counts per
    slot index for expert e (experts in original order)."""
    order = sorted(range(NUM_EXPERTS), key=lambda e: -loads[e])
    sl = [loads[e] for e in order]

    for sizes in ((496, 320, 240), (512, 384, 256), (512, 512, 384)):
        if sum(sizes) * 8 < sum(sl) or max(sl) > sum(sizes) * 8:
            continue
        cover = _try_cover(tuple(sizes), tuple(sl))
        if cover is not None:
            assign = [None] * NUM_EXPERTS
            for rank, e in enumerate(order):
                assign[e] = cover[rank]
            return tuple(sizes), assign

    # fallback: one expert per core, capacity = max load
    sizes = _capacity_chunks(max(loads))
    assign = [tuple([1] * len(sizes)) for _ in range(NUM_EXPERTS)]
    return sizes, assign


# ------------------------------------------------------------- device IR ----
def _build(chunks):
    """Per-core Bacc graph for slot sizes `chunks` (each /16, 240..512;
    the final slot's last phase-B sub-tile may have <128 partitions)."""
    import concourse.bacc as bacc
    import concourse.bass as bass
    import concourse.mybir as mybir
    import concourse.tile as tile

    n_chunks = len(chunks)
    DC = HIDDEN // 128  # 8 contraction chunks for x@w1
    FT = FFN // 128  # 32 f-tiles
    FG = FFN // 512  # 8 f-groups (512 wide)
    DO = HIDDEN // 512  # 2 output-d chunks
    CT = sum(chunks)
    XW = -(-CT // 16) * 16  # xT width padded so DMA lines stay 32B-aligned
    # phase-B sub-tiles, globally indexed; s is staged sub-major [k*128+p]
    subs = []  # (chunk_idx, t0, o, tsz, k)
    k = 0
    t0 = 0
    for t, c in enumerate(chunks):
        o = 0
        while o < c:
            tsz = min(128, c - o)
            subs.append((t, t0, o, tsz, k))
            k += 1
            o += tsz
        t0 += c
    N_SUB = k
    S_PAD = 128 * N_SUB

    bf16 = mybir.dt.bfloat16
    f32 = mybir.dt.float32

    nc = bacc.Bacc("TRN2", target_bir_lowering=False, debug=False,
                   num_devices=N_CORES)

    xT_d = nc.dram_tensor("xT", [HIDDEN, XW], bf16, kind="ExternalInput")
    # per-slot weights stacked on axis 0; w1/w3 host-pre-tiled [fg,p,dc,512]
    w1_d = nc.dram_tensor("w1", [n_chunks, FG, 128, DC, 512], bf16,
                          kind="ExternalInput")
    w3_d = nc.dram_tensor("w3", [n_chunks, FG, 128, DC, 512], bf16,
                          kind="ExternalInput")
    w2_d = nc.dram_tensor("w2", [n_chunks, FFN, HIDDEN], bf16,
                          kind="ExternalInput")
    s_d = nc.dram_tensor("s", [S_PAD], f32, kind="ExternalInput")
    y_d = nc.dram_tensor("y", [CT, HIDDEN], f32, kind="ExternalOutput")

    xT_v = xT_d.ap().rearrange("(dc p) c -> p dc c", p=128)
    w2_v = w2_d.ap().rearrange("n (ft p) d -> n p ft d", p=128)
    s_v = s_d.ap().rearrange("(j p) -> p j", p=128)

    with tile.TileContext(nc) as tc:
        with (
            tc.tile_pool(name="res", bufs=1) as res,
            tc.tile_pool(name="w13", bufs=4) as w13,
            tc.tile_pool(name="hp", bufs=1) as hp,
            tc.tile_pool(name="sil", bufs=4) as silp,
            tc.tile_pool(name="yo", bufs=4) as yop,
            tc.tile_pool(name="ps", bufs=2, space=bass.MemorySpace.PSUM) as ps,
            tc.tile_pool(name="yps", bufs=4, space=bass.MemorySpace.PSUM) as yps,
        ):
            # resident tensors.  xT is dc-split into two tiles so the first
            # phase-A accumulation (dc 0..3) only waits on half the load.
            xTa = res.tile([128, DC // 2, XW], bf16, tag="xTa")
            xTb = res.tile([128, DC // 2, XW], bf16, tag="xTb")
            w2 = res.tile([128, FT, HIDDEN], bf16, tag="w2")
            s_sb = res.tile([128, N_SUB], f32, tag="s")

            def xt(dc):
                return (xTa if dc < DC // 2 else xTb)[:, dc % (DC // 2), :]

            # HAM pre-warm: the first ~7us are framework preamble + DMA
            # ring priming during which the PE would idle cold; throwaway
            # matmuls keep the activity monitor busy so real matmuls start
            # at 2.4GHz, ending roughly when the first weights land.
            warm_sb = silp.tile([128, 128], bf16, tag="warm_in", bufs=1)
            nc.gpsimd.memset(warm_sb[:], 0.0)
            warm_ps = ps.tile([128, 128], f32, tag="ph1", name="warm_ps")
            for i in range(N_WARM):
                nc.tensor.matmul(warm_ps[:], warm_sb[:], warm_sb[:],
                                 start=(i == 0), stop=(i == N_WARM - 1))

            # startup-critical loads split in dc-quarters so they ride
            # separate DMA queues (per-queue BW during ring-prime is low)
            nc.sync.dma_start(xTa[:, 0:2, 0:chunks[0]],
                              xT_v[:, 0:2, 0:chunks[0]])
            nc.sync.dma_start(xTa[:, 2:4, 0:chunks[0]],
                              xT_v[:, 2:4, 0:chunks[0]])

            # ---- w1/w3 issue-ahead queue (3 f-groups deep) ----
            fg_steps = [(t, fg) for t in range(n_chunks) for fg in range(FG)]
            w13_fifo = []
            issued = [0]

            def issue_w13():
                t, fg = fg_steps[issued[0]]
                if t == 0 and fg == 0:
                    # first weights split in dc-quarters (separate tiles:
                    # Tile deps are tile-granular) so the first
                    # accumulation starts after minimal startup DMA and
                    # the transfers spread across DMA queues
                    w1a0 = w13.tile([128, 2, 512], bf16, tag="w1a0", bufs=1)
                    w1a1 = w13.tile([128, 2, 512], bf16, tag="w1a1", bufs=1)
                    w3a0 = w13.tile([128, 2, 512], bf16, tag="w3a0", bufs=1)
                    w3a1 = w13.tile([128, 2, 512], bf16, tag="w3a1", bufs=1)
                    w1b = w13.tile([128, 4, 512], bf16, tag="w1b", bufs=1)
                    w3b = w13.tile([128, 4, 512], bf16, tag="w3b", bufs=1)
                    nc.sync.dma_start(w1a0[:], w1_d.ap()[0][0][:, 0:2, :])
                    nc.sync.dma_start(w3a0[:], w3_d.ap()[0][0][:, 0:2, :])
                    nc.sync.dma_start(w1a1[:], w1_d.ap()[0][0][:, 2:4, :])
                    nc.sync.dma_start(w3a1[:], w3_d.ap()[0][0][:, 2:4, :])
                    nc.sync.dma_start(xTb[:, :, 0:chunks[0]],
                                      xT_v[:, DC // 2:DC, 0:chunks[0]])
                    nc.sync.dma_start(w1b[:], w1_d.ap()[0][0][:, 4:DC, :])
                    nc.sync.dma_start(w3b[:], w3_d.ap()[0][0][:, 4:DC, :])
                    w13_fifo.append(([(w1a0, 0), (w1a1, 2), (w1b, 4)],
                                     [(w3a0, 0), (w3a1, 2), (w3b, 4)]))
                else:
                    w1_sb = w13.tile([128, DC, 512], bf16, tag="w1")
                    w3_sb = w13.tile([128, DC, 512], bf16, tag="w3")
                    nc.sync.dma_start(w1_sb[:], w1_d.ap()[t][fg])
                    nc.sync.dma_start(w3_sb[:], w3_d.ap()[t][fg])
                    w13_fifo.append(([(w1_sb, 0)], [(w3_sb, 0)]))
                issued[0] += 1

            def _wslice(parts, dc):
                for tile_, base in parts:
                    if base <= dc < base + tile_.shape[1]:
                        return tile_[:, dc - base, :]
                raise AssertionError(dc)

            t0 = 0
            step = 0
            for t, chunk in enumerate(chunks):
                hT = hp.tile([128, FT, chunk], bf16, tag="hT")
                # ---- phase A: hT[f, t] for this slot ----
                for fg in range(FG):
                    while issued[0] <= min(step + 3, len(fg_steps) - 1):
                        issue_w13()
                    w1_parts, w3_parts = w13_fifo.pop(0)
                    step += 1
                    if t == 0:
                        if fg in (2, 3, 5, 7):
                            # stream slot-0 w2 strictly behind the critical
                            # early w1/w3 loads; lands before phase B
                            q = {2: 0, 3: 1, 5: 2, 7: 3}[fg]
                            nc.sync.dma_start(
                                w2[:, q * 8:(q + 1) * 8, :],
                                w2_v[0][:, q * 8:(q + 1) * 8, :])
                        if fg == 6:
                            nc.sync.dma_start(s_sb[:], s_v)
                        if fg == 4 and n_chunks > 1:
                            # prefetch the later slots' xT columns
                            o = chunks[0]
                            c = min(sum(chunks[1:3]), CT - o)
                            nc.sync.dma_start(
                                xTa[:, :, o:o + c],
                                xT_v[:, 0:DC // 2, o:o + c])
                            nc.sync.dma_start(
                                xTb[:, :, o:o + c],
                                xT_v[:, DC // 2:DC, o:o + c])
                    elif fg < 4:
                        # refill the w2 tile for THIS slot (released by
                        # phase B of slot t-1, which ended as this phase A
                        # began on the PE)
                        nc.sync.dma_start(
                            w2[:, fg * 8:(fg + 1) * 8, :],
                            w2_v[t][:, fg * 8:(fg + 1) * 8, :])
                    elif fg == 5 and t + 2 < n_chunks:
                        # slots 3+ xT prefetched one-ahead (fallback
                        # structures only; the primary plan has 3 slots)
                        tn = t + 2
                        o = sum(chunks[:tn])
                        c = chunks[tn]
                        nc.sync.dma_start(xTa[:, :, o:o + c],
                                          xT_v[:, 0:DC // 2, o:o + c])
                        nc.sync.dma_start(xTb[:, :, o:o + c],
                                          xT_v[:, DC // 2:DC, o:o + c])

                    for fl in range(4):
                        ft = fg * 4 + fl
                        ph1 = ps.tile([128, chunk], f32, tag="ph1")
                        ph3 = ps.tile([128, chunk], f32, tag="ph3")
                        for dc in range(DC):
                            nc.tensor.matmul(
                                ph1[:],
                                _wslice(w1_parts, dc)[:, fl * 128:(fl + 1) * 128],
                                xt(dc)[:, t0:t0 + chunk],
                                start=(dc == 0), stop=(dc == DC - 1),
                            )
                        for dc in range(DC):
                            nc.tensor.matmul(
                                ph3[:],
                                _wslice(w3_parts, dc)[:, fl * 128:(fl + 1) * 128],
                                xt(dc)[:, t0:t0 + chunk],
                                start=(dc == 0), stop=(dc == DC - 1),
                            )
                        sil = silp.tile([128, chunk], bf16, tag="sil")
                        nc.scalar.activation(
                            sil[:], ph1[:], mybir.ActivationFunctionType.Silu
                        )
                        nc.vector.tensor_mul(hT[:, ft, :], sil[:], ph3[:])

                # ---- phase B: y[t, d] for this slot ----
                csubs = [sb for sb in subs if sb[0] == t]
                for (_, _, o, tsz, kk) in csubs:
                    for do in range(DO):
                        is_tail = (t == n_chunks - 1
                                   and (o, tsz, kk) == (csubs[-1][2],
                                                        csubs[-1][3],
                                                        csubs[-1][4])
                                   and do == DO - 1)
                        if is_tail:
                            # very last output: split the f-accumulation in
                            # half so only one cheap fused multiply-add +
                            # store trails the final matmul
                            ypA = yps.tile([128, 512], f32, tag="yp",
                                           name="ypA")
                            for f in range(FT // 2):
                                nc.tensor.matmul(
                                    ypA[0:tsz, :],
                                    hT[:, f, o:o + tsz],
                                    w2[:, f, do * 512:(do + 1) * 512],
                                    start=(f == 0), stop=(f == FT // 2 - 1),
                                )
                            ysbA = yop.tile([128, 512], f32, tag="ysb")
                            nc.scalar.activation(
                                ysbA[0:tsz, :], ypA[0:tsz, :],
                                mybir.ActivationFunctionType.Copy,
                                scale=s_sb[0:tsz, kk:kk + 1],
                            )
                            ypB = yps.tile([128, 512], f32, tag="yp",
                                           name="ypB")
                            for f in range(FT // 2, FT):
                                nc.tensor.matmul(
                                    ypB[0:tsz, :],
                                    hT[:, f, o:o + tsz],
                                    w2[:, f, do * 512:(do + 1) * 512],
                                    start=(f == FT // 2), stop=(f == FT - 1),
                                )
                            ysb = yop.tile([128, 512], f32, tag="ysb")
                            nc.vector.scalar_tensor_tensor(
                                ysb[0:tsz, :], ypB[0:tsz, :],
                                s_sb[0:tsz, kk:kk + 1], ysbA[0:tsz, :],
                                mybir.AluOpType.mult, mybir.AluOpType.add,
                            )
                            nc.sync.dma_start(
                                y_d[t0 + o:t0 + o + tsz,
                                    do * 512:(do + 1) * 512],
                                ysb[0:tsz, :],
                            )
                            continue
                        yp = yps.tile([128, 512], f32, tag="yp",
                                      name=f"yp{kk}_{do}")
                        for f in range(FT):
                            nc.tensor.matmul(
                                yp[0:tsz, :],
                                hT[:, f, o:o + tsz],
                                w2[:, f, do * 512:(do + 1) * 512],
                                start=(f == 0), stop=(f == FT - 1),
                            )
                        ysb = yop.tile([128, 512], f32, tag="ysb")
                        # scale on ScalarE: out = in * s (per-partition)
                        nc.scalar.activation(
                            ysb[0:tsz, :], yp[0:tsz, :],
                            mybir.ActivationFunctionType.Copy,
                            scale=s_sb[0:tsz, kk:kk + 1],
                        )
                        nc.sync.dma_start(
                            y_d[t0 + o:t0 + o + tsz,
                                do * 512:(do + 1) * 512],
                            ysb[0:tsz, :],
                        )
                t0 += chunk
    nc.compile()
    return nc


def _get_nc(chunks):
    if chunks not in _nc_cache:
        _nc_cache[chunks] = _build(chunks)
    return _nc_cache[chunks]


def _pretile_w13(w):
    """[HIDDEN, FFN] -> [fg, p, dc, 512] bf16 (see _build's w1_d layout)."""
    w4 = np.ascontiguousarray(w).reshape(HIDDEN // 128, 128, FFN // 512, 512)
    return np.ascontiguousarray(w4.transpose(2, 1, 0, 3)).astype(_BF16)


# ---------------------------------------------------------------- kernel ----
def kernel(hidden_states, gate_w, gate_b, w1, w3, w2, _trace=False):
    from concourse.bass_utils import run_bass_kernel_spmd

    B, S, D = hidden_states.shape
    T = B * S
    x = np.asarray(hidden_states, np.float32).reshape(T, D)
    idxs, wts = _route(x, np.asarray(gate_w, np.float32),
                       np.asarray(gate_b, np.float32))
    loads = [len(i) for i in idxs]
    sizes, assign = _plan(loads)
    n_slots = len(sizes)
    CT = sum(sizes)
    XW = -(-CT // 16) * 16
    slot_off = [sum(sizes[:j]) for j in range(n_slots)]
    # global sub-tile count for the s layout
    n_sub = sum(-(-c // 128) for c in sizes)
    nc = _get_nc(tuple(sizes))

    # instance pools per slot index: (core) list
    pools = [list(range(N_CORES)) for _ in range(n_slots)]
    # per-core, per-slot: (expert, tok_array, wt_array)
    placement = [[None] * n_slots for _ in range(N_CORES)]
    for e in range(NUM_EXPERTS):
        tok, wt = idxs[e], wts[e]
        pos = 0
        for j in range(n_slots):
            for _ in range(assign[e][j]):
                core = pools[j].pop()
                take = max(0, min(sizes[j], len(tok) - pos))
                placement[core][j] = (e, tok[pos:pos + take],
                                      wt[pos:pos + take])
                pos += take
        assert pos >= len(tok), (e, pos, len(tok))

    w1 = np.asarray(w1)
    w3 = np.asarray(w3)
    w2 = np.asarray(w2)
    w1p = [_pretile_w13(w1[e]) for e in range(NUM_EXPERTS)]
    w3p = [_pretile_w13(w3[e]) for e in range(NUM_EXPERTS)]
    w2p = [np.ascontiguousarray(w2[e]).astype(_BF16)
           for e in range(NUM_EXPERTS)]

    in_maps = []
    for core in range(N_CORES):
        xT = np.zeros((D, XW), _BF16)
        w_cols = np.zeros((CT,), np.float32)
        im = {"xT": xT}
        slot_e = []
        for j in range(n_slots):
            pl = placement[core][j]
            e = pl[0] if pl is not None else 0
            slot_e.append(e)
            if pl is not None and len(pl[1]):
                tok, wt = pl[1], pl[2]
                xT[:, slot_off[j]:slot_off[j] + len(tok)] = \
                    x[tok].T.astype(_BF16)
                w_cols[slot_off[j]:slot_off[j] + len(tok)] = wt
        im["w1"] = np.stack([w1p[e] for e in slot_e])
        im["w3"] = np.stack([w3p[e] for e in slot_e])
        im["w2"] = np.stack([w2p[e] for e in slot_e])
        # s staged sub-major: s[k*128 + p] = weight of sub-tile k, row p
        s = np.zeros((128 * n_sub,), np.float32)
        k = 0
        for j, c in enumerate(sizes):
            o = 0
            while o < c:
                tsz = min(128, c - o)
                s[k * 128:k * 128 + tsz] = \
                    w_cols[slot_off[j] + o:slot_off[j] + o + tsz]
                k += 1
                o += tsz
        im["s"] = s
        in_maps.append(im)

    res = run_bass_kernel_spmd(nc, in_maps, core_ids=list(range(N_CORES)),
                               trace=_trace)

    out = np.zeros((T, D), np.float32)
    for core in range(N_CORES):
        y = res.results[core]["y"]
        for j in range(n_slots):
            pl = placement[core][j]
            if pl is None or not len(pl[1]):
                continue
            tok = pl[1]
            out[tok] += y[slot_off[j]:slot_off[j] + len(tok)]
    out = out.reshape(B, S, D)
    if _trace:
        return out, res
    return out


# revision 11
# speedup vs baseline: 1.0002x; 1.0002x over previous
"""Trainium2 Bass kernel for nn_BlockSparseMoE (top-2 of 8 experts, SwiGLU).

Strategy (chunk-scheduled expert parallelism):
  - Host: compute router (gate matmul + softmax + top-2 + renorm) in fp64.
  - The device graph processes a fixed list of token-chunk slots per core
    (identical across cores).  Each slot streams its OWN w1/w3/w2 from
    DRAM, so the host is free to bind ANY expert's weights to any slot.
    Capacity therefore isn't max-expert-load (1129 for the reference
    router) but the optimum of a small bin-covering problem: slot sizes
    (496, 320, 240) x 8 cores cover the 8 expert loads with ~3% padding
    (sum 1056/core), vs 10% for one-expert-per-core.
  - Device (SPMD x8): per slot, hT[f, t] = silu(x@w1)^T * (x@w3)^T is
    computed directly transposed (phase A), then y[t, d] = hT^T @ w2
    scaled by the renormalized top-2 weight (phase B). bf16 in, fp32 PSUM.
  - Host: scatter-add the per-slot outputs back by token.

Per-core layout notes:
  - w1/w3 arrive host-pre-tiled [fg, p, dc, 512] so each f-group DMA is a
    contiguous 8KB line per partition; streamed per slot with a 3-deep
    issue-ahead queue (the 240-token slot consumes 2MB per 6.4us f-group
    window -- deeper prefetch rides through that burst).
  - w2 is ONE resident SBUF tile, re-filled per slot during phase A of
    that slot (phase B of slot t-1 releases it exactly when phase A of
    slot t starts on the PE, and phase A lasts >= 51us vs 22us of DMA).
"""

import numpy as np
import ml_dtypes

HIDDEN = 1024
FFN = 4096
NUM_EXPERTS = 8
TOP_K = 2
N_CORES = 8
N_WARM = 72

_BF16 = ml_dtypes.bfloat16
_nc_cache = {}


# ---------------------------------------------------------------- router ----
def _route(x, gate_w, gate_b):
    """Top-2 routing. Returns per-expert (token_idx, renorm_weight)."""
    logits = x.astype(np.float64) @ gate_w.astype(np.float64) + gate_b.astype(
        np.float64
    )
    logits -= logits.max(axis=-1, keepdims=True)
    p = np.exp(logits)
    p /= p.sum(axis=-1, keepdims=True)
    # top-2 by prob, ties broken by lower index (matches jax.lax.top_k)
    top2 = np.argsort(-p, axis=-1, kind="stable")[:, :TOP_K]
    pt = np.take_along_axis(p, top2, axis=-1)
    wt = pt / pt.sum(axis=-1, keepdims=True)
    idxs, wts = [], []
    for e in range(NUM_EXPERTS):
        mask = top2 == e  # [T, 2]
        tok = np.nonzero(mask.any(axis=-1))[0]
        w = wt[tok, np.argmax(mask[tok], axis=-1)]
        idxs.append(tok)
        wts.append(w.astype(np.float32))
    return idxs, wts


# ------------------------------------------------------------ slot planner --
def _try_cover(sizes, loads, node_budget=200000):
    """Can 8 instances of each slot size cover the loads (sum >= load each)?
    Returns per-load tuples (count per slot size) or None."""
    import itertools
    import functools

    ns = len(sizes)
    nodes = [0]

    @functools.lru_cache(maxsize=None)
    def combos_for(need, rem):
        out = []
        maxk = [min(rem[j], (need // sizes[j]) + 1) for j in range(ns)]
        for ks in itertools.product(*[range(k + 1) for k in maxk]):
            tot = sum(k * s for k, s in zip(ks, sizes))
            # minimal covers only (dropping any instance goes below need)
            if tot >= need and all(
                k == 0 or tot - s < need for k, s in zip(ks, sizes)
            ):
                out.append(ks)
        out.sort(key=lambda ks: sum(k * s for k, s in zip(ks, sizes)))
        return out

    result = []

    def dfs(ei, rem):
        nodes[0] += 1
        if nodes[0] > node_budget:
            return False
        if ei == len(loads):
            return True
        for ks in combos_for(loads[ei], rem):
            result.append(ks)
            if dfs(ei + 1, tuple(r - k for r, k in zip(rem, ks))):
                return True
            result.pop()
        return False

    if dfs(0, tuple([8] * ns)):
        return list(result)
    return None


def _capacity_chunks(max_load):
    """Fallback slot structure: per-core chunks covering one full expert
    (the original one-expert-per-core layout). Always feasible."""
    C = max_load
    n = max(1, -(-C // 512))
    chunks = []
    rem = C
    for i in range(n - 1):
        c = min(512, -(-rem // ((n - i) * 128)) * 128)
        chunks.append(c)
        rem -= c
    while n > 1 and rem < 240 and chunks:
        for i in range(len(chunks)):
            if rem >= 240:
                break
            if chunks[i] > 256:
                chunks[i] -= 128
                rem += 128
        else:
            break
    rem = -(-rem // 16) * 16  # pad ragged tail to /16 (padding cols, s=0)
    chunks.append(rem)
    return tuple(chunks)


def _plan(loads):
    """Choose slot sizes + per-expert slot counts for the 8-core fleet.

    Returns (sizes, assign) where assign[e] = tuple of instance counts per
    slot index for expert e (experts in original order)."""
    order = sorted(range(NUM_EXPERTS), key=lambda e: -loads[e])
    sl = [loads[e] for e in order]

    for sizes in ((496, 320, 240), (512, 384, 256), (512, 512, 384)):
        if sum(sizes) * 8 < sum(sl) or max(sl) > sum(sizes) * 8:
            continue
        cover = _try_cover(tuple(sizes), tuple(sl))
        if cover is not None:
            assign = [None] * NUM_EXPERTS
            for rank, e in enumerate(order):
                assign[e] = cover[rank]
            return tuple(sizes), assign

    # fallback: one expert per core, capacity = max load
    sizes = _capacity_chunks(max(loads))
    assign = [tuple([1] * len(sizes)) for _ in range(NUM_EXPERTS)]
    return sizes, assign


# ------------------------------------------------------------- device IR ----
def _build(chunks):
    """Per-core Bacc graph for slot sizes `chunks` (each /16, 240..512;
    the final slot's last phase-B sub-tile may have <128 partitions)."""
    import concourse.bacc as bacc
    import concourse.bass as bass
    import concourse.mybir as mybir
    import concourse.tile as tile

    n_chunks = len(chunks)
    DC = HIDDEN // 128  # 8 contraction chunks for x@w1
    FT = FFN // 128  # 32 f-tiles
    FG = FFN // 512  # 8 f-groups (512 wide)
    DO = HIDDEN // 512  # 2 output-d chunks
    CT = sum(chunks)
    XW = -(-CT // 16) * 16  # xT width padded so DMA lines stay 32B-aligned
    # phase-B sub-tiles, globally indexed; s is staged sub-major [k*128+p]
    subs = []  # (chunk_idx, t0, o, tsz, k)
    k = 0
    t0 = 0
    for t, c in enumerate(chunks):
        o = 0
        while o < c:
            tsz = min(128, c - o)
            subs.append((t, t0, o, tsz, k))
            k += 1
            o += tsz
        t0 += c
    N_SUB = k
    S_PAD = 128 * N_SUB

    bf16 = mybir.dt.bfloat16
    f32 = mybir.dt.float32

    nc = bacc.Bacc("TRN2", target_bir_lowering=False, debug=False,
                   num_devices=N_CORES)

    xT_d = nc.dram_tensor("xT", [HIDDEN, XW], bf16, kind="ExternalInput")
    # per-slot weights stacked on axis 0; w1/w3 host-pre-tiled [fg,p,dc,512]
    w1_d = nc.dram_tensor("w1", [n_chunks, FG, 128, DC, 512], bf16,
                          kind="ExternalInput")
    w3_d = nc.dram_tensor("w3", [n_chunks, FG, 128, DC, 512], bf16,
                          kind="ExternalInput")
    w2_d = nc.dram_tensor("w2", [n_chunks, FFN, HIDDEN], bf16,
                          kind="ExternalInput")
    s_d = nc.dram_tensor("s", [S_PAD], f32, kind="ExternalInput")
    y_d = nc.dram_tensor("y", [CT, HIDDEN], f32, kind="ExternalOutput")

    xT_v = xT_d.ap().rearrange("(dc p) c -> p dc c", p=128)
    w2_v = w2_d.ap().rearrange("n (ft p) d -> n p ft d", p=128)
    s_v = s_d.ap().rearrange("(j p) -> p j", p=128)

    with tile.TileContext(nc) as tc:
        with (
            tc.tile_pool(name="res", bufs=1) as res,
            tc.tile_pool(name="w13", bufs=4) as w13,
            tc.tile_pool(name="hp", bufs=1) as hp,
            tc.tile_pool(name="sil", bufs=4) as silp,
            tc.tile_pool(name="yo", bufs=4) as yop,
            tc.tile_pool(name="ps", bufs=2, space=bass.MemorySpace.PSUM) as ps,
            tc.tile_pool(name="yps", bufs=4, space=bass.MemorySpace.PSUM) as yps,
        ):
            # resident tensors.  xT is dc-split into two tiles so the first
            # phase-A accumulation (dc 0..3) only waits on half the load.
            xTa = res.tile([128, DC // 2, XW], bf16, tag="xTa")
            xTb = res.tile([128, DC // 2, XW], bf16, tag="xTb")
            w2 = res.tile([128, FT, HIDDEN], bf16, tag="w2")
            s_sb = res.tile([128, N_SUB], f32, tag="s")

            def xt(dc):
                return (xTa if dc < DC // 2 else xTb)[:, dc % (DC // 2), :]

            # HAM pre-warm: the first ~7us are framework preamble + DMA
            # ring priming during which the PE would idle cold; throwaway
            # matmuls keep the activity monitor busy so real matmuls start
            # at 2.4GHz, ending roughly when the first weights land.
            warm_sb = silp.tile([128, 128], bf16, tag="warm_in", bufs=1)
            nc.gpsimd.memset(warm_sb[:], 0.0)
            warm_ps = ps.tile([128, 128], f32, tag="ph1", name="warm_ps")
            for i in range(N_WARM):
                nc.tensor.matmul(warm_ps[:], warm_sb[:], warm_sb[:],
                                 start=(i == 0), stop=(i == N_WARM - 1))

            # startup-critical loads split in dc-quarters so they ride
            # separate DMA queues (per-queue BW during ring-prime is low)
            nc.sync.dma_start(xTa[:, 0:2, 0:chunks[0]],
                              xT_v[:, 0:2, 0:chunks[0]])
            nc.sync.dma_start(xTa[:, 2:4, 0:chunks[0]],
                              xT_v[:, 2:4, 0:chunks[0]])

            # ---- w1/w3 issue-ahead queue (3 f-groups deep) ----
            fg_steps = [(t, fg) for t in range(n_chunks) for fg in range(FG)]
            w13_fifo = []
            issued = [0]

            def issue_w13():
                t, fg = fg_steps[issued[0]]
                if t == 0 and fg == 0:
                    # first weights split in dc-quarters (separate tiles:
                    # Tile deps are tile-granular) so the first
                    # accumulation starts after minimal startup DMA and
                    # the transfers spread across DMA queues
                    w1a0 = w13.tile([128, 2, 512], bf16, tag="w1a0", bufs=1)
                    w1a1 = w13.tile([128, 2, 512], bf16, tag="w1a1", bufs=1)
                    w3a0 = w13.tile([128, 2, 512], bf16, tag="w3a0", bufs=1)
                    w3a1 = w13.tile([128, 2, 512], bf16, tag="w3a1", bufs=1)
                    w1b = w13.tile([128, 4, 512], bf16, tag="w1b", bufs=1)
                    w3b = w13.tile([128, 4, 512], bf16, tag="w3b", bufs=1)
                    nc.sync.dma_start(w1a0[:], w1_d.ap()[0][0][:, 0:2, :])
                    nc.sync.dma_start(w3a0[:], w3_d.ap()[0][0][:, 0:2, :])
                    nc.sync.dma_start(w1a1[:], w1_d.ap()[0][0][:, 2:4, :])
                    nc.sync.dma_start(w3a1[:], w3_d.ap()[0][0][:, 2:4, :])
                    nc.sync.dma_start(xTb[:, :, 0:chunks[0]],
                                      xT_v[:, DC // 2:DC, 0:chunks[0]])
                    nc.sync.dma_start(w1b[:], w1_d.ap()[0][0][:, 4:DC, :])
                    nc.sync.dma_start(w3b[:], w3_d.ap()[0][0][:, 4:DC, :])
                    w13_fifo.append(([(w1a0, 0), (w1a1, 2), (w1b, 4)],
                                     [(w3a0, 0), (w3a1, 2), (w3b, 4)]))
                else:
                    w1_sb = w13.tile([128, DC, 512], bf16, tag="w1")
                    w3_sb = w13.tile([128, DC, 512], bf16, tag="w3")
                    nc.sync.dma_start(w1_sb[:], w1_d.ap()[t][fg])
                    nc.sync.dma_start(w3_sb[:], w3_d.ap()[t][fg])
                    w13_fifo.append(([(w1_sb, 0)], [(w3_sb, 0)]))
                issued[0] += 1

            def _wslice(parts, dc):
                for tile_, base in parts:
                    if base <= dc < base + tile_.shape[1]:
                        return tile_[:, dc - base, :]
                raise AssertionError(dc)

            t0 = 0
            step = 0
            for t, chunk in enumerate(chunks):
                hT = hp.tile([128, FT, chunk], bf16, tag="hT")
                # ---- phase A: hT[f, t] for this slot ----
                for fg in range(FG):
                    while issued[0] <= min(step + 3, len(fg_steps) - 1):
                        issue_w13()
                    w1_parts, w3_parts = w13_fifo.pop(0)
                    step += 1
                    if t == 0:
                        if fg in (2, 3, 5, 7):
                            # stream slot-0 w2 strictly behind the critical
                            # early w1/w3 loads; lands before phase B
                            q = {2: 0, 3: 1, 5: 2, 7: 3}[fg]
                            nc.sync.dma_start(
                                w2[:, q * 8:(q + 1) * 8, :],
                                w2_v[0][:, q * 8:(q + 1) * 8, :])
                        if fg == 6:
                            nc.sync.dma_start(s_sb[:], s_v)
                        if fg == 4 and n_chunks > 1:
                            # prefetch the later slots' xT columns
                            o = chunks[0]
                            c = min(sum(chunks[1:3]), CT - o)
                            nc.sync.dma_start(
                                xTa[:, :, o:o + c],
                                xT_v[:, 0:DC // 2, o:o + c])
                            nc.sync.dma_start(
                                xTb[:, :, o:o + c],
                                xT_v[:, DC // 2:DC, o:o + c])
                    elif fg < 4:
                        # refill the w2 tile for THIS slot (released by
                        # phase B of slot t-1, which ended as this phase A
                        # began on the PE)
                        nc.sync.dma_start(
                            w2[:, fg * 8:(fg + 1) * 8, :],
                            w2_v[t][:, fg * 8:(fg + 1) * 8, :])
                    elif fg == 5 and t + 2 < n_chunks:
                        # slots 3+ xT prefetched one-ahead (fallback
                        # structures only; the primary plan has 3 slots)
                        tn = t + 2
                        o = sum(chunks[:tn])
                        c = chunks[tn]
                        nc.sync.dma_start(xTa[:, :, o:o + c],
                                          xT_v[:, 0:DC // 2, o:o + c])
                        nc.sync.dma_start(xTb[:, :, o:o + c],
                                          xT_v[:, DC // 2:DC, o:o + c])

                    def mm_acc(ph, parts, fl, dcs, start, stop):
                        for i, dc in enumerate(dcs):
                            nc.tensor.matmul(
                                ph[:],
                                _wslice(parts, dc)[:, fl * 128:(fl + 1) * 128],
                                xt(dc)[:, t0:t0 + chunk],
                                start=(start and i == 0),
                                stop=(stop and i == len(dcs) - 1),
                                skip_group_check=True,
                            )

                    def sil_mul(ph1, ph3, ft):
                        sil = silp.tile([128, chunk], bf16, tag="sil")
                        nc.scalar.activation(
                            sil[:], ph1[:], mybir.ActivationFunctionType.Silu
                        )
                        nc.vector.tensor_mul(hT[:, ft, :], sil[:], ph3[:])

                    if t == 0 and fg == 0:
                        # startup special: fl0/fl1 accumulate dc0..3 first
                        # (16 matmuls that need only the early quarter
                        # tiles), giving the w1b/w3b/xTb transfers ~3.3us
                        # more time to land before any matmul depends on
                        # them -- keeps the PE stream gapless through the
                        # DMA ramp so the HAM never re-throttles.
                        half = []
                        for fl in (0, 1):
                            ph1 = ps.tile([128, chunk], f32, tag="ph1")
                            ph3 = ps.tile([128, chunk], f32, tag="ph3")
                            mm_acc(ph1, w1_parts, fl, range(4), True, False)
                            mm_acc(ph3, w3_parts, fl, range(4), True, False)
                            half.append((fl, ph1, ph3))
                        for fl, ph1, ph3 in half:
                            mm_acc(ph1, w1_parts, fl, range(4, DC), False, True)
                            mm_acc(ph3, w3_parts, fl, range(4, DC), False, True)
                            sil_mul(ph1, ph3, fl)
                        rest_fl = (2, 3)
                    else:
                        rest_fl = range(4)
                    for fl in rest_fl:
                        ft = fg * 4 + fl
                        ph1 = ps.tile([128, chunk], f32, tag="ph1")
                        ph3 = ps.tile([128, chunk], f32, tag="ph3")
                        mm_acc(ph1, w1_parts, fl, range(DC), True, True)
                        mm_acc(ph3, w3_parts, fl, range(DC), True, True)
                        sil_mul(ph1, ph3, ft)

                # ---- phase B: y[t, d] for this slot ----
                csubs = [sb for sb in subs if sb[0] == t]
                for (_, _, o, tsz, kk) in csubs:
                    for do in range(DO):
                        is_tail = (t == n_chunks - 1
                                   and (o, tsz, kk) == (csubs[-1][2],
                                                        csubs[-1][3],
                                                        csubs[-1][4])
                                   and do == DO - 1)
                        if is_tail:
                            # very last output: split the f-accumulation in
                            # half so only one cheap fused multiply-add +
                            # store trails the final matmul
                            ypA = yps.tile([128, 512], f32, tag="yp",
                                           name="ypA")
                            for f in range(FT // 2):
                                nc.tensor.matmul(
                                    ypA[0:tsz, :],
                                    hT[:, f, o:o + tsz],
                                    w2[:, f, do * 512:(do + 1) * 512],
                                    start=(f == 0), stop=(f == FT // 2 - 1),
                                )
                            ysbA = yop.tile([128, 512], f32, tag="ysb")
                            nc.scalar.activation(
                                ysbA[0:tsz, :], ypA[0:tsz, :],
                                mybir.ActivationFunctionType.Copy,
                                scale=s_sb[0:tsz, kk:kk + 1],
                            )
                            ypB = yps.tile([128, 512], f32, tag="yp",
                                           name="ypB")
                            for f in range(FT // 2, FT):
                                nc.tensor.matmul(
                                    ypB[0:tsz, :],
                                    hT[:, f, o:o + tsz],
                                    w2[:, f, do * 512:(do + 1) * 512],
                                    start=(f == FT // 2), stop=(f == FT - 1),
                                )
                            ysb = yop.tile([128, 512], f32, tag="ysb")
                            nc.vector.scalar_tensor_tensor(
                                ysb[0:tsz, :], ypB[0:tsz, :],
                                s_sb[0:tsz, kk:kk + 1], ysbA[0:tsz, :],
                                mybir.AluOpType.mult, mybir.AluOpType.add,
                            )
                            nc.sync.dma_start(
                                y_d[t0 + o:t0 + o + tsz,
                                    do * 512:(do + 1) * 512],
                                ysb[0:tsz, :],
                            )
                            continue
                        yp = yps.tile([128, 512], f32, tag="yp",
                                      name=f"yp{kk}_{do}")
                        for f in range(FT):
                            nc.tensor.matmul(
                                yp[0:tsz, :],
                                hT[:, f, o:o + tsz],
                                w2[:, f, do * 512:(do + 1) * 512],
                                start=(f == 0), stop=(f == FT - 1),
                            )
                        ysb = yop.tile([128, 512], f32, tag="ysb")
                        # scale on ScalarE: out = in * s (per-partition)
                        nc.scalar.activation(
                            ysb[0:tsz, :], yp[0:tsz, :],
                            mybir.ActivationFunctionType.Copy,
                            scale=s_sb[0:tsz, kk:kk + 1],
                        )
                        nc.sync.dma_start(
                            y_d[t0 + o:t0 + o + tsz,
                                do * 512:(do + 1) * 512],
                            ysb[0:tsz, :],
                        )
                t0 += chunk
    nc.compile()
    return nc


def _get_nc(chunks):
    if chunks not in _nc_cache:
        _nc_cache[chunks] = _build(chunks)
    return _nc_cache[chunks]


def _pretile_w13(w):
    """[HIDDEN, FFN] -> [fg, p, dc, 512] bf16 (see _build's w1_d layout)."""
    w4 = np.ascontiguousarray(w).reshape(HIDDEN // 128, 128, FFN // 512, 512)
    return np.ascontiguousarray(w4.transpose(2, 1, 0, 3)).astype(_BF16)


# ---------------------------------------------------------------- kernel ----
def kernel(hidden_states, gate_w, gate_b, w1, w3, w2, _trace=False):
    from concourse.bass_utils import run_bass_kernel_spmd

    B, S, D = hidden_states.shape
    T = B * S
    x = np.asarray(hidden_states, np.float32).reshape(T, D)
    idxs, wts = _route(x, np.asarray(gate_w, np.float32),
                       np.asarray(gate_b, np.float32))
    loads = [len(i) for i in idxs]
    sizes, assign = _plan(loads)
    n_slots = len(sizes)
    CT = sum(sizes)
    XW = -(-CT // 16) * 16
    slot_off = [sum(sizes[:j]) for j in range(n_slots)]
    # global sub-tile count for the s layout
    n_sub = sum(-(-c // 128) for c in sizes)
    nc = _get_nc(tuple(sizes))

    # instance pools per slot index: (core) list
    pools = [list(range(N_CORES)) for _ in range(n_slots)]
    # per-core, per-slot: (expert, tok_array, wt_array)
    placement = [[None] * n_slots for _ in range(N_CORES)]
    for e in range(NUM_EXPERTS):
        tok, wt = idxs[e], wts[e]
        pos = 0
        for j in range(n_slots):
            for _ in range(assign[e][j]):
                core = pools[j].pop()
                take = max(0, min(sizes[j], len(tok) - pos))
                placement[core][j] = (e, tok[pos:pos + take],
                                      wt[pos:pos + take])
                pos += take
        assert pos >= len(tok), (e, pos, len(tok))

    w1 = np.asarray(w1)
    w3 = np.asarray(w3)
    w2 = np.asarray(w2)
    w1p = [_pretile_w13(w1[e]) for e in range(NUM_EXPERTS)]
    w3p = [_pretile_w13(w3[e]) for e in range(NUM_EXPERTS)]
    w2p = [np.ascontiguousarray(w2[e]).astype(_BF16)
           for e in range(NUM_EXPERTS)]

    in_maps = []
    for core in range(N_CORES):
        xT = np.zeros((D, XW), _BF16)
        w_cols = np.zeros((CT,), np.float32)
        im = {"xT": xT}
        slot_e = []
        for j in range(n_slots):
            pl = placement[core][j]
            e = pl[0] if pl is not None else 0
            slot_e.append(e)
            if pl is not None and len(pl[1]):
                tok, wt = pl[1], pl[2]
                xT[:, slot_off[j]:slot_off[j] + len(tok)] = \
                    x[tok].T.astype(_BF16)
                w_cols[slot_off[j]:slot_off[j] + len(tok)] = wt
        im["w1"] = np.stack([w1p[e] for e in slot_e])
        im["w3"] = np.stack([w3p[e] for e in slot_e])
        im["w2"] = np.stack([w2p[e] for e in slot_e])
        # s staged sub-major: s[k*128 + p] = weight of sub-tile k, row p
        s = np.zeros((128 * n_sub,), np.float32)
        k = 0
        for j, c in enumerate(sizes):
            o = 0
            while o < c:
                tsz = min(128, c - o)
                s[k * 128:k * 128 + tsz] = \
                    w_cols[slot_off[j] + o:slot_off[j] + o + tsz]
                k += 1
                o += tsz
        im["s"] = s
        in_maps.append(im)

    res = run_bass_kernel_spmd(nc, in_maps, core_ids=list(range(N_CORES)),
                               trace=_trace)

    out = np.zeros((T, D), np.float32)
    for core in range(N_CORES):
        y = res.results[core]["y"]
        for j in range(n_slots):
            pl = placement[core][j]
            if pl is None or not len(pl[1]):
                continue
            tok = pl[1]
            out[tok] += y[slot_off[j]:slot_off[j] + len(tok)]
    out = out.reshape(B, S, D)
    if _trace:
        return out, res
    return out
